# revision 3
# baseline (speedup 1.0000x reference)
"""Trainium2 kernel for BIMBlockND (nn_APUNet_33079838114069).

The reference computes, per batch n:
    xf = im2col(x)                      # (C*P, K*K) with P=256 patches
    out = g_weight @ xf  (1x1 conv)     # (8192, 64)
    scatter-back + residual add

Folding all batches into one GEMM column dim and the residual into the
weights (W' = g + I), the whole problem is:

    Out[8192, 1024] = (g + I) @ Xf,   Xf[i, n*64+s] = im2col(x)

Sharding: tensor-parallel over the 8192 output rows across 8 cores
(1024 rows each).  Every core gets the full Xf (host-side im2col) and
its W'^T shard; no collectives needed.  Compute in bf16 (1 cycle/row on
the PE vs 4 for f32; rel-err ~2e-3, well under the 2e-2 gate).
"""

import numpy as np
import ml_dtypes

B, C, H, W = 16, 32, 128, 128
K = 8
HP = WP = 16
P = HP * WP          # 256 patches
CI = C * P           # 8192 channels in GEMM space
NCORES = 8
MS = CI // NCORES    # 1024 output rows per core
NCOL = B * K * K     # 1024 GEMM columns (batch x intra-patch position)
PTILE = 128          # partition tile
NTILE = 512          # psum bank free size (f32)
KT = CI // PTILE     # 64 k-tiles
MT = MS // PTILE     # 8 m-tiles
NB = NCOL // NTILE   # 2 n-blocks

_NC = None


def _build_nc():
    from concourse import bacc, tile
    import concourse.mybir as mybir

    nc = bacc.Bacc("TRN2", target_bir_lowering=False, debug=False,
                   num_devices=NCORES)
    wt = nc.declare_dram_parameter("wt", [CI, MS], mybir.dt.bfloat16,
                                   isOutput=False)
    xf = nc.declare_dram_parameter("xf", [CI, NCOL], mybir.dt.bfloat16,
                                   isOutput=False)
    out = nc.declare_dram_parameter("out", [MS, NCOL], mybir.dt.float32,
                                    isOutput=True)

    bf16 = mybir.dt.bfloat16
    f32 = mybir.dt.float32
    with tile.TileContext(nc) as tc:
        with (
            tc.tile_pool(name="wtp", bufs=1) as wtp,
            tc.tile_pool(name="xfp", bufs=6) as xfp,
            tc.tile_pool(name="xf1p", bufs=1) as xf1p,
            tc.tile_pool(name="outp", bufs=4) as outp,
            tc.tile_pool(name="warmp", bufs=1) as warmp,
            tc.tile_pool(name="pp", bufs=1, space="PSUM") as pp,
        ):
            # --- PE warm-up: dummy matmuls on memset tiles during the
            # initial DMA wait flip the HAM clock gate to 2.4 GHz before
            # the real matmul stream starts. ---
            warm_w = warmp.tile([PTILE, PTILE], bf16, name="warm_w",
                                tag="warm_w")
            warm_x = warmp.tile([PTILE, NTILE], bf16, name="warm_x",
                                tag="warm_x")
            nc.gpsimd.memset(warm_w[:], 0.0)
            nc.gpsimd.memset(warm_x[:], 0.0)
            warm_ps = pp.tile([PTILE, NTILE], f32, name="warm_ps", tag="ps0")
            for i in range(10):
                nc.tensor.matmul(warm_ps[:], warm_w[:], warm_x[:],
                                 start=True, stop=True)

            wt_tiles = [None] * KT
            xf1_tiles = [None] * KT
            half = MS // 2
            # --- pass 0 (columns 0:512): kt outer / m inner, streaming.
            # wt halves go on the scalar-engine DGE queue, xf on sync, so
            # the first tiles land (and the first matmul starts) sooner.
            # The pass-1 xf half is prefetched here too. ---
            psums0 = [pp.tile([PTILE, NTILE], f32, name=f"ps0_{m}",
                              tag=f"ps{m}") for m in range(MT)]
            for kt in range(KT):
                r0, r1 = kt * PTILE, (kt + 1) * PTILE
                wt_tiles[kt] = wtp.tile([PTILE, MS], bf16,
                                        name=f"wt_{kt}", tag=f"wt{kt}")
                nc.scalar.dma_start(wt_tiles[kt][:, :half],
                                    wt[r0:r1, :half])
                nc.scalar.dma_start(wt_tiles[kt][:, half:],
                                    wt[r0:r1, half:])
                xt = xfp.tile([PTILE, NTILE], bf16,
                              name=f"xf0_{kt}", tag="xf", bufs=6)
                nc.sync.dma_start(xt[:], xf[r0:r1, 0:NTILE])
                xf1_tiles[kt] = xf1p.tile([PTILE, NTILE], bf16,
                                          name=f"xf1_{kt}", tag=f"xf1{kt}")
                nc.sync.dma_start(xf1_tiles[kt][:],
                                  xf[r0:r1, NTILE:2 * NTILE])
                for m in range(MT):
                    nc.tensor.matmul(
                        psums0[m][:],
                        wt_tiles[kt][:, m * PTILE:(m + 1) * PTILE],
                        xt[:],
                        start=(kt == 0),
                        stop=(kt == KT - 1),
                    )
            for m in range(MT):
                ot = outp.tile([PTILE, NTILE], f32, name=f"o0_{m}",
                               tag="o", bufs=4)
                nc.vector.tensor_copy(ot[:], psums0[m][:])
                nc.sync.dma_start(out[m * PTILE:(m + 1) * PTILE, 0:NTILE],
                                  ot[:])
            # --- pass 1 (columns 512:1024): m outer / kt inner on the
            # prefetched resident xf half, so each psum's copy+store
            # overlaps the next m's matmuls and the kernel tail is one
            # copy + one DMA instead of eight. ---
            for m in range(MT):
                ps = pp.tile([PTILE, NTILE], f32, name=f"ps1_{m}",
                             tag=f"ps{m}")
                for kt in range(KT):
                    nc.tensor.matmul(
                        ps[:],
                        wt_tiles[kt][:, m * PTILE:(m + 1) * PTILE],
                        xf1_tiles[kt][:],
                        start=(kt == 0),
                        stop=(kt == KT - 1),
                    )
                ot = outp.tile([PTILE, NTILE], f32, name=f"o1_{m}",
                               tag="o", bufs=4)
                nc.vector.tensor_copy(ot[:], ps[:])
                nc.sync.dma_start(out[m * PTILE:(m + 1) * PTILE, NTILE:],
                                  ot[:])
    nc.finalize()
    return nc


def _get_nc():
    global _NC
    if _NC is None:
        _NC = _build_nc()
    return _NC


def _make_in_maps(x, g_weight):
    x = np.asarray(x, dtype=np.float32)
    g = np.asarray(g_weight, dtype=np.float32)
    # Xf[(c,ph,pw), (n,kr,kc)] = x[n, c, ph*8+kr, pw*8+kc]
    xp = x.reshape(B, C, HP, K, WP, K).transpose(1, 2, 4, 0, 3, 5)
    Xf = np.ascontiguousarray(xp.reshape(CI, NCOL))
    # W'^T with the residual folded in: W'[o,i] = g[o,i] + (o==i)
    WT = g.T.copy()
    idx = np.arange(CI)
    WT[idx, idx] += 1.0
    WTb = WT.astype(ml_dtypes.bfloat16)
    Xfb = np.ascontiguousarray(Xf.astype(ml_dtypes.bfloat16))
    return [
        {"wt": np.ascontiguousarray(WTb[:, r * MS:(r + 1) * MS]), "xf": Xfb}
        for r in range(NCORES)
    ]


def _assemble(results):
    Out = np.concatenate([results[r]["out"] for r in range(NCORES)], axis=0)
    o6 = Out.reshape(C, HP, WP, B, K, K).transpose(3, 0, 1, 4, 2, 5)
    return np.ascontiguousarray(o6.reshape(B, C, H, W)).astype(np.float32)


def kernel(x, g_weight):
    from concourse.bass_utils import run_bass_kernel_spmd
    nc = _get_nc()
    in_maps = _make_in_maps(x, g_weight)
    res = run_bass_kernel_spmd(nc, in_maps, core_ids=list(range(NCORES)))
    return _assemble(res.results)


def kernel_timed(x, g_weight, **kwargs):
    """Like kernel() but with neuron-profile tracing; returns (out, res)."""
    from concourse.bass_utils import run_bass_kernel_spmd
    nc = _get_nc()
    in_maps = _make_in_maps(x, g_weight)
    res = run_bass_kernel_spmd(nc, in_maps, core_ids=list(range(NCORES)),
                               trace=True, **kwargs)
    return _assemble(res.results), res


# revision 4
# speedup vs baseline: 1.0593x; 1.0593x over previous
"""Trainium2 kernel for BIMBlockND (nn_APUNet_33079838114069).

The reference computes, per batch n:
    xf = im2col(x)                      # (C*P, K*K) with P=256 patches
    out = g_weight @ xf  (1x1 conv)     # (8192, 64)
    scatter-back + residual add

Folding all batches into one GEMM column dim and the residual into the
weights (W' = g + I), the whole problem is:

    Out[8192, 1024] = (g + I) @ Xf,   Xf[i, n*64+s] = im2col(x)

Sharding: tensor-parallel over the 8192 output rows across 8 cores
(1024 rows each).  Every core gets the full Xf (host-side im2col) and
its W'^T shard; no collectives needed.  Compute in bf16 (1 cycle/row on
the PE vs 4 for f32; rel-err ~2e-3, well under the 2e-2 gate).
"""

import numpy as np
import ml_dtypes

B, C, H, W = 16, 32, 128, 128
K = 8
HP = WP = 16
P = HP * WP          # 256 patches
CI = C * P           # 8192 channels in GEMM space
NCORES = 8
MS = CI // NCORES    # 1024 output rows per core
NCOL = B * K * K     # 1024 GEMM columns (batch x intra-patch position)
PTILE = 128          # partition tile
NTILE = 512          # psum bank free size (f32)
KT = CI // PTILE     # 64 k-tiles
MT = MS // PTILE     # 8 m-tiles
NB = NCOL // NTILE   # 2 n-blocks

_NC = None


def _build_nc():
    from concourse import bacc, tile
    import concourse.mybir as mybir

    nc = bacc.Bacc("TRN2", target_bir_lowering=False, debug=False,
                   num_devices=NCORES)
    wt = nc.declare_dram_parameter("wt", [CI, MS], mybir.dt.bfloat16,
                                   isOutput=False)
    xf = nc.declare_dram_parameter("xf", [CI, NCOL], mybir.dt.bfloat16,
                                   isOutput=False)
    out = nc.declare_dram_parameter("out", [MS, NCOL], mybir.dt.float32,
                                    isOutput=True)

    bf16 = mybir.dt.bfloat16
    f32 = mybir.dt.float32
    with tile.TileContext(nc) as tc:
        with (
            tc.tile_pool(name="wtp", bufs=1) as wtp,
            tc.tile_pool(name="xfp", bufs=6) as xfp,
            tc.tile_pool(name="xf1p", bufs=1) as xf1p,
            tc.tile_pool(name="outp", bufs=4) as outp,
            tc.tile_pool(name="warmp", bufs=1) as warmp,
            tc.tile_pool(name="pp", bufs=1, space="PSUM") as pp,
        ):
            # --- PE warm-up: dummy matmuls on memset tiles during the
            # initial DMA wait flip the HAM clock gate to 2.4 GHz before
            # the real matmul stream starts. ---
            warm_w = warmp.tile([PTILE, PTILE], bf16, name="warm_w",
                                tag="warm_w")
            warm_x = warmp.tile([PTILE, NTILE], bf16, name="warm_x",
                                tag="warm_x")
            nc.gpsimd.memset(warm_w[:], 0.0)
            nc.gpsimd.memset(warm_x[:], 0.0)
            warm_ps = pp.tile([PTILE, NTILE], f32, name="warm_ps", tag="ps0")
            for i in range(10):
                nc.tensor.matmul(warm_ps[:], warm_w[:], warm_x[:],
                                 start=True, stop=True)

            wt_tiles = [None] * KT
            half = MS // 2
            # Both passes kt-outer / m-inner, streaming xf.  wt halves go
            # on the scalar-engine DGE queue (two parallel queues halve
            # the first tile's latency), xf on sync.
            for nb in range(NB):
                psums = [pp.tile([PTILE, NTILE], f32, name=f"ps_{nb}_{m}",
                                 tag=f"ps{m}") for m in range(MT)]
                for kt in range(KT):
                    r0, r1 = kt * PTILE, (kt + 1) * PTILE
                    if nb == 0:
                        wt_tiles[kt] = wtp.tile([PTILE, MS], bf16,
                                                name=f"wt_{kt}",
                                                tag=f"wt{kt}")
                        nc.scalar.dma_start(wt_tiles[kt][:, :half],
                                            wt[r0:r1, :half])
                        nc.scalar.dma_start(wt_tiles[kt][:, half:],
                                            wt[r0:r1, half:])
                    xt = xfp.tile([PTILE, NTILE], bf16,
                                  name=f"xf_{nb}_{kt}", tag="xf", bufs=6)
                    nc.sync.dma_start(
                        xt[:], xf[r0:r1, nb * NTILE:(nb + 1) * NTILE])
                    for m in range(MT):
                        nc.tensor.matmul(
                            psums[m][:],
                            wt_tiles[kt][:, m * PTILE:(m + 1) * PTILE],
                            xt[:],
                            start=(kt == 0),
                            stop=(kt == KT - 1),
                        )
                for m in range(MT):
                    ot = outp.tile([PTILE, NTILE], f32, name=f"o_{nb}_{m}",
                                   tag="o", bufs=4)
                    nc.vector.tensor_copy(ot[:], psums[m][:])
                    nc.sync.dma_start(
                        out[m * PTILE:(m + 1) * PTILE,
                            nb * NTILE:(nb + 1) * NTILE],
                        ot[:])
    nc.finalize()
    return nc


def _get_nc():
    global _NC
    if _NC is None:
        _NC = _build_nc()
    return _NC


def _make_in_maps(x, g_weight):
    x = np.asarray(x, dtype=np.float32)
    g = np.asarray(g_weight, dtype=np.float32)
    # Xf[(c,ph,pw), (n,kr,kc)] = x[n, c, ph*8+kr, pw*8+kc]
    xp = x.reshape(B, C, HP, K, WP, K).transpose(1, 2, 4, 0, 3, 5)
    Xf = np.ascontiguousarray(xp.reshape(CI, NCOL))
    # W'^T with the residual folded in: W'[o,i] = g[o,i] + (o==i)
    WT = g.T.copy()
    idx = np.arange(CI)
    WT[idx, idx] += 1.0
    WTb = WT.astype(ml_dtypes.bfloat16)
    Xfb = np.ascontiguousarray(Xf.astype(ml_dtypes.bfloat16))
    return [
        {"wt": np.ascontiguousarray(WTb[:, r * MS:(r + 1) * MS]), "xf": Xfb}
        for r in range(NCORES)
    ]


def _assemble(results):
    Out = np.concatenate([results[r]["out"] for r in range(NCORES)], axis=0)
    o6 = Out.reshape(C, HP, WP, B, K, K).transpose(3, 0, 1, 4, 2, 5)
    return np.ascontiguousarray(o6.reshape(B, C, H, W)).astype(np.float32)


def kernel(x, g_weight):
    from concourse.bass_utils import run_bass_kernel_spmd
    nc = _get_nc()
    in_maps = _make_in_maps(x, g_weight)
    res = run_bass_kernel_spmd(nc, in_maps, core_ids=list(range(NCORES)))
    return _assemble(res.results)


def kernel_timed(x, g_weight, **kwargs):
    """Like kernel() but with neuron-profile tracing; returns (out, res)."""
    from concourse.bass_utils import run_bass_kernel_spmd
    nc = _get_nc()
    in_maps = _make_in_maps(x, g_weight)
    res = run_bass_kernel_spmd(nc, in_maps, core_ids=list(range(NCORES)),
                               trace=True, **kwargs)
    return _assemble(res.results), res


# revision 5
# speedup vs baseline: 1.1864x; 1.1199x over previous
"""Trainium2 kernel for BIMBlockND (nn_APUNet_33079838114069).

The reference computes, per batch n:
    xf = im2col(x)                      # (C*P, K*K) with P=256 patches
    out = g_weight @ xf  (1x1 conv)     # (8192, 64)
    scatter-back + residual add

Folding all batches into one GEMM column dim and the residual into the
weights (W' = g + I), the whole problem is:

    Out[8192, 1024] = (g + I) @ Xf,   Xf[i, n*64+s] = im2col(x)

Sharding: tensor-parallel over the 8192 output rows across 8 cores
(1024 rows each).  Every core gets the full Xf (host-side im2col) and
its W'^T shard; no collectives needed.  Compute in bf16 (1 cycle/row on
the PE vs 4 for f32; rel-err ~2e-3, well under the 2e-2 gate).
"""

import numpy as np
import ml_dtypes

B, C, H, W = 16, 32, 128, 128
K = 8
HP = WP = 16
P = HP * WP          # 256 patches
CI = C * P           # 8192 channels in GEMM space
NCORES = 8
MS = CI // NCORES    # 1024 output rows per core
NCOL = B * K * K     # 1024 GEMM columns (batch x intra-patch position)
PTILE = 128          # partition tile
NTILE = 512          # psum bank free size (f32)
KT = CI // PTILE     # 64 k-tiles
MT = MS // PTILE     # 8 m-tiles
NB = NCOL // NTILE   # 2 n-blocks

_NC = None


def _build_nc():
    from concourse import bacc, tile
    import concourse.mybir as mybir

    nc = bacc.Bacc("TRN2", target_bir_lowering=False, debug=False,
                   num_devices=NCORES)
    wt = nc.declare_dram_parameter("wt", [CI, MS], mybir.dt.bfloat16,
                                   isOutput=False)
    xf = nc.declare_dram_parameter("xf", [CI, NCOL], mybir.dt.bfloat16,
                                   isOutput=False)
    out = nc.declare_dram_parameter("out", [MS, NCOL], mybir.dt.float32,
                                    isOutput=True)

    bf16 = mybir.dt.bfloat16
    f32 = mybir.dt.float32
    with tile.TileContext(nc) as tc:
        with (
            tc.tile_pool(name="wtp", bufs=1) as wtp,
            tc.tile_pool(name="xfp", bufs=6) as xfp,
            tc.tile_pool(name="xf1p", bufs=1) as xf1p,
            tc.tile_pool(name="outp", bufs=4) as outp,
            tc.tile_pool(name="warmp", bufs=1) as warmp,
            tc.tile_pool(name="pp", bufs=1, space="PSUM") as pp,
        ):
            # --- PE warm-up: dummy matmuls on memset tiles during the
            # initial DMA wait flip the HAM clock gate to 2.4 GHz before
            # the real matmul stream starts. ---
            warm_w = warmp.tile([PTILE, PTILE], bf16, name="warm_w",
                                tag="warm_w")
            warm_x = warmp.tile([PTILE, NTILE], bf16, name="warm_x",
                                tag="warm_x")
            nc.gpsimd.memset(warm_w[:], 0.0)
            nc.gpsimd.memset(warm_x[:], 0.0)
            warm_ps = pp.tile([PTILE, NTILE], f32, name="warm_ps", tag="ps0")
            for i in range(10):
                nc.tensor.matmul(warm_ps[:], warm_w[:], warm_x[:],
                                 start=True, stop=True)

            wt_tiles = [None] * KT
            half = MS // 2
            # Both passes kt-outer / m-inner, streaming xf.  wt halves go
            # on the scalar-engine DGE queue (two parallel queues halve
            # the first tile's latency), xf on sync.
            for nb in range(NB):
                psums = [pp.tile([PTILE, NTILE], f32, name=f"ps_{nb}_{m}",
                                 tag=f"ps{m}") for m in range(MT)]
                for kt in range(KT):
                    r0, r1 = kt * PTILE, (kt + 1) * PTILE
                    if nb == 0:
                        wt_tiles[kt] = wtp.tile([PTILE, MS], bf16,
                                                name=f"wt_{kt}",
                                                tag=f"wt{kt}")
                        nc.sync.dma_start(wt_tiles[kt][:], wt[r0:r1, :])
                    xt = xfp.tile([PTILE, NTILE], bf16,
                                  name=f"xf_{nb}_{kt}", tag="xf", bufs=6)
                    nc.sync.dma_start(
                        xt[:], xf[r0:r1, nb * NTILE:(nb + 1) * NTILE])
                    for m in range(MT):
                        nc.tensor.matmul(
                            psums[m][:],
                            wt_tiles[kt][:, m * PTILE:(m + 1) * PTILE],
                            xt[:],
                            start=(kt == 0),
                            stop=(kt == KT - 1),
                        )
                for m in range(MT):
                    ot = outp.tile([PTILE, NTILE], f32, name=f"o_{nb}_{m}",
                                   tag="o", bufs=4)
                    nc.vector.tensor_copy(ot[:], psums[m][:])
                    nc.sync.dma_start(
                        out[m * PTILE:(m + 1) * PTILE,
                            nb * NTILE:(nb + 1) * NTILE],
                        ot[:])
    nc.finalize()
    return nc


def _get_nc():
    global _NC
    if _NC is None:
        _NC = _build_nc()
    return _NC


def _make_in_maps(x, g_weight):
    x = np.asarray(x, dtype=np.float32)
    g = np.asarray(g_weight, dtype=np.float32)
    # Xf[(c,ph,pw), (n,kr,kc)] = x[n, c, ph*8+kr, pw*8+kc]
    xp = x.reshape(B, C, HP, K, WP, K).transpose(1, 2, 4, 0, 3, 5)
    Xf = np.ascontiguousarray(xp.reshape(CI, NCOL))
    # W'^T with the residual folded in: W'[o,i] = g[o,i] + (o==i)
    WT = g.T.copy()
    idx = np.arange(CI)
    WT[idx, idx] += 1.0
    WTb = WT.astype(ml_dtypes.bfloat16)
    Xfb = np.ascontiguousarray(Xf.astype(ml_dtypes.bfloat16))
    return [
        {"wt": np.ascontiguousarray(WTb[:, r * MS:(r + 1) * MS]), "xf": Xfb}
        for r in range(NCORES)
    ]


def _assemble(results):
    Out = np.concatenate([results[r]["out"] for r in range(NCORES)], axis=0)
    o6 = Out.reshape(C, HP, WP, B, K, K).transpose(3, 0, 1, 4, 2, 5)
    return np.ascontiguousarray(o6.reshape(B, C, H, W)).astype(np.float32)


def kernel(x, g_weight):
    from concourse.bass_utils import run_bass_kernel_spmd
    nc = _get_nc()
    in_maps = _make_in_maps(x, g_weight)
    res = run_bass_kernel_spmd(nc, in_maps, core_ids=list(range(NCORES)))
    return _assemble(res.results)


def kernel_timed(x, g_weight, **kwargs):
    """Like kernel() but with neuron-profile tracing; returns (out, res)."""
    from concourse.bass_utils import run_bass_kernel_spmd
    nc = _get_nc()
    in_maps = _make_in_maps(x, g_weight)
    res = run_bass_kernel_spmd(nc, in_maps, core_ids=list(range(NCORES)),
                               trace=True, **kwargs)
    return _assemble(res.results), res


# revision 6
# speedup vs baseline: 1.1888x; 1.0021x over previous
"""Trainium2 kernel for BIMBlockND (nn_APUNet_33079838114069).

The reference computes, per batch n:
    xf = im2col(x)                      # (C*P, K*K) with P=256 patches
    out = g_weight @ xf  (1x1 conv)     # (8192, 64)
    scatter-back + residual add

Folding all batches into one GEMM column dim and the residual into the
weights (W' = g + I), the whole problem is:

    Out[8192, 1024] = (g + I) @ Xf,   Xf[i, n*64+s] = im2col(x)

Sharding: tensor-parallel over the 8192 output rows across 8 cores
(1024 rows each).  Every core gets the full Xf (host-side im2col) and
its W'^T shard; no collectives needed.  Compute in bf16 (1 cycle/row on
the PE vs 4 for f32; rel-err ~2e-3, well under the 2e-2 gate).
"""

import numpy as np
import ml_dtypes

B, C, H, W = 16, 32, 128, 128
K = 8
HP = WP = 16
P = HP * WP          # 256 patches
CI = C * P           # 8192 channels in GEMM space
NCORES = 8
MS = CI // NCORES    # 1024 output rows per core
NCOL = B * K * K     # 1024 GEMM columns (batch x intra-patch position)
PTILE = 128          # partition tile
NTILE = 512          # psum bank free size (f32)
KT = CI // PTILE     # 64 k-tiles
MT = MS // PTILE     # 8 m-tiles
NB = NCOL // NTILE   # 2 n-blocks

_NC = None


def _build_nc():
    from concourse import bacc, tile
    import concourse.mybir as mybir

    nc = bacc.Bacc("TRN2", target_bir_lowering=False, debug=False,
                   num_devices=NCORES)
    wt = nc.declare_dram_parameter("wt", [CI, MS], mybir.dt.bfloat16,
                                   isOutput=False)
    xf = nc.declare_dram_parameter("xf", [CI, NCOL], mybir.dt.bfloat16,
                                   isOutput=False)
    out = nc.declare_dram_parameter("out", [MS, NCOL], mybir.dt.float32,
                                    isOutput=True)

    bf16 = mybir.dt.bfloat16
    f32 = mybir.dt.float32
    with tile.TileContext(nc) as tc:
        with (
            tc.tile_pool(name="wtp", bufs=1) as wtp,
            tc.tile_pool(name="xfp", bufs=6) as xfp,
            tc.tile_pool(name="xf1p", bufs=1) as xf1p,
            tc.tile_pool(name="outp", bufs=4) as outp,
            tc.tile_pool(name="warmp", bufs=1) as warmp,
            tc.tile_pool(name="pp", bufs=1, space="PSUM") as pp,
        ):
            # --- PE warm-up: dummy matmuls on memset tiles during the
            # initial DMA wait flip the HAM clock gate to 2.4 GHz before
            # the real matmul stream starts. ---
            warm_w = warmp.tile([PTILE, PTILE], bf16, name="warm_w",
                                tag="warm_w")
            warm_x = warmp.tile([PTILE, NTILE], bf16, name="warm_x",
                                tag="warm_x")
            nc.gpsimd.memset(warm_w[:], 0.0)
            nc.gpsimd.memset(warm_x[:], 0.0)
            warm_ps = pp.tile([PTILE, NTILE], f32, name="warm_ps", tag="ps0")
            for i in range(8):
                nc.tensor.matmul(warm_ps[:], warm_w[:], warm_x[:],
                                 start=True, stop=True)

            wt_tiles = [None] * KT
            half = MS // 2
            # Both passes kt-outer / m-inner, streaming xf.  wt halves go
            # on the scalar-engine DGE queue (two parallel queues halve
            # the first tile's latency), xf on sync.
            for nb in range(NB):
                psums = [pp.tile([PTILE, NTILE], f32, name=f"ps_{nb}_{m}",
                                 tag=f"ps{m}") for m in range(MT)]
                for kt in range(KT):
                    r0, r1 = kt * PTILE, (kt + 1) * PTILE
                    if nb == 0:
                        wt_tiles[kt] = wtp.tile([PTILE, MS], bf16,
                                                name=f"wt_{kt}",
                                                tag=f"wt{kt}")
                        nc.sync.dma_start(wt_tiles[kt][:], wt[r0:r1, :])
                    xt = xfp.tile([PTILE, NTILE], bf16,
                                  name=f"xf_{nb}_{kt}", tag="xf", bufs=6)
                    nc.sync.dma_start(
                        xt[:], xf[r0:r1, nb * NTILE:(nb + 1) * NTILE])
                    for m in range(MT):
                        nc.tensor.matmul(
                            psums[m][:],
                            wt_tiles[kt][:, m * PTILE:(m + 1) * PTILE],
                            xt[:],
                            start=(kt == 0),
                            stop=(kt == KT - 1),
                        )
                for m in range(MT):
                    ot = outp.tile([PTILE, NTILE], f32, name=f"o_{nb}_{m}",
                                   tag="o", bufs=4)
                    nc.vector.tensor_copy(ot[:], psums[m][:])
                    nc.sync.dma_start(
                        out[m * PTILE:(m + 1) * PTILE,
                            nb * NTILE:(nb + 1) * NTILE],
                        ot[:])
    nc.finalize()
    return nc


def _get_nc():
    global _NC
    if _NC is None:
        _NC = _build_nc()
    return _NC


def _make_in_maps(x, g_weight):
    x = np.asarray(x, dtype=np.float32)
    g = np.asarray(g_weight, dtype=np.float32)
    # Xf[(c,ph,pw), (n,kr,kc)] = x[n, c, ph*8+kr, pw*8+kc]
    xp = x.reshape(B, C, HP, K, WP, K).transpose(1, 2, 4, 0, 3, 5)
    Xf = np.ascontiguousarray(xp.reshape(CI, NCOL))
    # W'^T with the residual folded in: W'[o,i] = g[o,i] + (o==i)
    WT = g.T.copy()
    idx = np.arange(CI)
    WT[idx, idx] += 1.0
    WTb = WT.astype(ml_dtypes.bfloat16)
    Xfb = np.ascontiguousarray(Xf.astype(ml_dtypes.bfloat16))
    return [
        {"wt": np.ascontiguousarray(WTb[:, r * MS:(r + 1) * MS]), "xf": Xfb}
        for r in range(NCORES)
    ]


def _assemble(results):
    Out = np.concatenate([results[r]["out"] for r in range(NCORES)], axis=0)
    o6 = Out.reshape(C, HP, WP, B, K, K).transpose(3, 0, 1, 4, 2, 5)
    return np.ascontiguousarray(o6.reshape(B, C, H, W)).astype(np.float32)


def kernel(x, g_weight):
    from concourse.bass_utils import run_bass_kernel_spmd
    nc = _get_nc()
    in_maps = _make_in_maps(x, g_weight)
    res = run_bass_kernel_spmd(nc, in_maps, core_ids=list(range(NCORES)))
    return _assemble(res.results)


def kernel_timed(x, g_weight, **kwargs):
    """Like kernel() but with neuron-profile tracing; returns (out, res)."""
    from concourse.bass_utils import run_bass_kernel_spmd
    nc = _get_nc()
    in_maps = _make_in_maps(x, g_weight)
    res = run_bass_kernel_spmd(nc, in_maps, core_ids=list(range(NCORES)),
                               trace=True, **kwargs)
    return _assemble(res.results), res
